# revision 40
# baseline (speedup 1.0000x reference)
"""Multi-head causal attention (B=4, T=2048, D=1024, H=16, Dh=64) on 8 NeuronCores.

Sharding: tensor-parallel over heads. Core c owns heads (2c, 2c+1):
  - qkv projection columns for those heads (W_qkv slice, 1024x384)
  - out projection rows for those heads (W_out slice, 128x1024)
  - x is replicated (host pre-transposes to (1024, 8192) so all device DMAs
    are contiguous)
Each core produces a partial (8192, 1024) bf16 output; the host sums the 8
partials in fp32.

v2 scheduling: the whole kernel is one continuously-fed PE stream. The
Tensor engine on TRN2 ramps to 2.4 GHz only after ~3us of uninterrupted
execution (half rate otherwise), so all stalls matter double. Projection
work for later batches and the deferred out-projection of earlier tq-blocks
are held in queues of small closures and "pumped" into the attention loop
between S->exp->PV steps, paced by a PE-cost ledger, so the PE never waits
on the exp (ACT) chain. Other changes vs v1:
  - head B's PV psum lands on partitions 63..127 (vaug packs [ones|v] for
    head B, [v|ones] for head A) so ctx for both heads is built in place and
    the out-projection contracts K=128 without any SBUF->SBUF partition
    shift.
  - 1/l reciprocal and the ctx scale multiply read the PV psum directly
    (no intermediate copies); tri-mask multiply runs on GpSimd.
  - diagonal S tiles only compute/exp/stream columns >= lo (the strictly
    causal-zero region is never written or read: PV accumulates into a
    column subrange, which is exact because those P entries are zero).
  - output partials are written bf16 (host sums in fp32).
"""

import os
import sys

sys.path.insert(0, "/opt/trn_rl_repo")

from contextlib import ExitStack

import numpy as np

import concourse.bass as bass
import concourse.tile as tile
from concourse import bacc, mybir
from concourse.bass_utils import run_bass_kernel_spmd

F32 = mybir.dt.float32
AF = mybir.ActivationFunctionType

B, T, D = 4, 2048, 1024
H, DH = 16, 64
BT = B * T  # 8192
N_CORES = 8
HEADS_PER_CORE = H // N_CORES  # 2
FEATS = HEADS_PER_CORE * DH  # 128 features per core
TQB = 512  # tq block size (one psum bank of fp32)
N_TQB = T // TQB  # 4 per batch
N_TK = T // 128  # 16 tk tiles per batch
DCH = D // 128  # 8 d-model chunks

# PE-cost estimates (ns at 2.4 GHz) used only for pacing the interleave.
C_PROJ_GROUP = 1700.0  # 8 matmuls N=512
C_VTRANS = 400.0
C_OUTPROJ_SLICE = 430.0  # 2 matmuls N=512
C_DMA = 100.0
ITER_PE = 1100.0  # S pair (~213) + PV pair (~427) + margin
# DVE cannot move data across partitions on real HW (sim allows it); the
# head-B ctx shift must be a SBUF->SBUF DMA.
SHIFT_DVE = os.environ.get("SHIFT", "dma") == "dve"
OUTPROJ_DELAY = int(os.environ.get("OPD", "12"))  # iterations before eligible
DBG = os.environ.get("DBG", "0") == "1"


def build_kernel(mm_dtype=mybir.dt.bfloat16):
    MDT = mm_dtype
    nc = bacc.Bacc(
        "TRN2", target_bir_lowering=False, debug=False, num_devices=N_CORES
    )

    x_t = nc.declare_dram_parameter("x_t", [D, BT], MDT, isOutput=False)
    wqkv = nc.declare_dram_parameter("wqkv", [D, 3 * FEATS], MDT, isOutput=False)
    wout = nc.declare_dram_parameter("wout", [FEATS, D], MDT, isOutput=False)
    tri = nc.declare_dram_parameter("tri", [128, 128], MDT, isOutput=False)
    ident = nc.declare_dram_parameter("ident", [128, 128], MDT, isOutput=False)
    out = nc.declare_dram_parameter("out", [BT, D], MDT, isOutput=True)
    if DBG:
        dbg_qT = nc.declare_dram_parameter("dbg_qT", [128, T], MDT, isOutput=True)
        dbg_kT = nc.declare_dram_parameter("dbg_kT", [128, T], MDT, isOutput=True)
        dbg_vaug = nc.declare_dram_parameter(
            "dbg_vaug", [128, N_TK * 132], MDT, isOutput=True
        )
        dbg_pt = nc.declare_dram_parameter("dbg_pt", [128, 1024], MDT, isOutput=True)
        dbg_ops = nc.declare_dram_parameter("dbg_ops", [65, 1024], F32, isOutput=True)
        dbg_lr = nc.declare_dram_parameter("dbg_lr", [2, TQB], F32, isOutput=True)
        dbg_ctx = nc.declare_dram_parameter("dbg_ctx", [128, TQB], MDT, isOutput=True)

    with tile.TileContext(nc) as tc, ExitStack() as ctx:
        const = ctx.enter_context(tc.tile_pool(name="const", bufs=1))
        xt_pool = ctx.enter_context(tc.tile_pool(name="xt", bufs=6))
        qk_pool = ctx.enter_context(tc.tile_pool(name="qk", bufs=6))
        vt_pool = ctx.enter_context(tc.tile_pool(name="vt", bufs=2))
        vaug_pool = ctx.enter_context(tc.tile_pool(name="vaug", bufs=3))
        pt_pool = ctx.enter_context(tc.tile_pool(name="pt", bufs=6))
        # 5 bufs: with OUTPROJ_DELAY~12 up to 4 ctx_packs are alive at once
        # (deferred slices still reading while new blocks write); ring=3
        # makes reuse race the out-proj LDW reads (NaN at OPD>=14)
        ctx_pool = ctx.enter_context(tc.tile_pool(name="ctx", bufs=5))
        osb_pool = ctx.enter_context(tc.tile_pool(name="osb", bufs=3))
        lr_pool = ctx.enter_context(tc.tile_pool(name="lr", bufs=4))
        bc_pool = ctx.enter_context(tc.tile_pool(name="bc", bufs=2))
        s_ps = ctx.enter_context(tc.tile_pool(name="s_ps", bufs=2, space="PSUM"))
        o_ps = ctx.enter_context(tc.tile_pool(name="o_ps", bufs=2, space="PSUM"))
        pj_ps = ctx.enter_context(tc.tile_pool(name="pj_ps", bufs=2, space="PSUM"))

        # --- constants (DMAs emitted below, interleaved with the first xt
        # load, so the first proj matmul waits minimally) ---
        wqkv_sb = const.tile([128, DCH, 3 * FEATS], MDT)
        wout_sb = const.tile([FEATS, D], MDT)
        tri_sb = const.tile([128, 128], MDT)
        ident_sb = const.tile([128, 128], MDT)

        def emit_const_dmas():
            # k/v columns first (group(0,1) needs kT soonest), then the
            # small constants
            wr = wqkv.rearrange("(c p) f -> p c f", p=128)
            nc.sync.dma_start(
                out=wqkv_sb[:, :, FEATS : 3 * FEATS],
                in_=wr[:, :, FEATS : 3 * FEATS],
            )
            nc.sync.dma_start(out=tri_sb[:], in_=tri[:])
            nc.sync.dma_start(out=ident_sb[:], in_=ident[:])
            nc.sync.dma_start(out=wout_sb[:], in_=wout[:])

        # ---------------- interleave machinery ----------------
        proj_q = []  # [(batch, cost, fn)] in dependency order
        outp_q = []  # [(eligible_iter, cost, fn)] deferred out-proj slices
        reserve_q = []  # [(cost, fn)] slices held back as tail filler
        credit = [0.0]  # PE-cost ledger for even spreading
        it_count = [0]  # global attention iteration counter

        def outp_ready():
            return outp_q and outp_q[0][0] <= it_count[0]

        def head_cost():
            if outp_ready():
                return outp_q[0][1]
            if proj_q:
                return proj_q[0][1]
            return None

        def pump_step(allowance):
            # accumulate credit; pop pieces while affordable
            credit[0] += allowance
            while True:
                hc = head_cost()
                if hc is None or credit[0] < hc:
                    break
                if outp_ready():
                    _, cost, fn = outp_q.pop(0)
                else:
                    _, cost, fn = proj_q.pop(0)
                fn()
                credit[0] -= cost

        def flush_proj(b):
            # everything batch b's attention depends on must be emitted
            while proj_q and proj_q[0][0] <= b:
                _, _, fn = proj_q.pop(0)
                fn()

        def queued_cost():
            return sum(c for _, c, _ in outp_q) + sum(c for _, c, _ in proj_q)

        # ---------------- projection chunks ----------------
        def make_proj_pieces(b):
            """qT/kT/v-aug production for batch b as (batch, cost, fn) pieces."""
            t0 = b * T
            qT = qk_pool.tile([128, T], MDT, tag="qT")  # 2 heads stacked on P
            kT = qk_pool.tile([128, T], MDT, tag="kT")
            # vaug columns: [0:64]=v_A, 64=ones_A, [65:129]=v_B, 129=ones_B
            vaug = vaug_pool.tile([128, N_TK, 132], MDT)
            pieces = []
            cell = {}

            def dma_piece(tqb, first=False, split=False):
                def fn():
                    if first:
                        nc.vector.memset(vaug[:, :, 64:65], 1.0)
                        nc.vector.memset(vaug[:, :, 129:130], 1.0)
                    xt = xt_pool.tile([128, DCH, TQB], MDT)
                    src = x_t[
                        :, t0 + tqb * TQB : t0 + (tqb + 1) * TQB
                    ].rearrange("(c p) t -> p c t", p=128)
                    if split:
                        # halve the first x tile's transfer so the first
                        # projection matmuls wait on 512KB, not 1MB
                        h = DCH // 2
                        nc.sync.dma_start(out=xt[:, 0:h], in_=src[:, 0:h])
                        nc.sync.dma_start(out=xt[:, h:DCH], in_=src[:, h:DCH])
                    else:
                        nc.sync.dma_start(out=xt[:], in_=src)
                    cell[tqb] = xt

                return fn

            def proj_group(tqb, g, next_dma=None):
                def fn():
                    if next_dma is not None:
                        next_dma()
                    ps = pj_ps.tile([128, TQB], F32, tag="pj")
                    xt = cell[tqb]
                    for ci in range(DCH):
                        nc.tensor.matmul(
                            ps[:],
                            wqkv_sb[:, ci, g * FEATS : (g + 1) * FEATS],
                            xt[:, ci, :],
                            start=(ci == 0),
                            stop=(ci == DCH - 1),
                        )
                    if g == 0:
                        nc.vector.tensor_copy(
                            qT[:, tqb * TQB : (tqb + 1) * TQB], ps[:]
                        )
                    elif g == 1:
                        nc.vector.tensor_copy(
                            kT[:, tqb * TQB : (tqb + 1) * TQB], ps[:]
                        )
                    else:
                        vt = vt_pool.tile([128, TQB], MDT)
                        nc.vector.tensor_copy(vt[:], ps[:])
                        cell[("v", tqb)] = vt

                return fn

            def vtrans_piece(tqb):
                def fn():
                    vt = cell[("v", tqb)]
                    for s in range(TQB // 128):
                        tp = pj_ps.tile([128, 128], MDT, tag="pj")
                        nc.tensor.transpose(
                            tp[:], vt[:, s * 128 : (s + 1) * 128], ident_sb[:]
                        )
                        tk = tqb * (TQB // 128) + s
                        # cols {0..63} U {65..128} <- heads A,B of tp
                        nc.vector.tensor_copy(
                            vaug[:, tk, 0:130].rearrange(
                                "p (g c) -> p g c", c=65
                            )[:, :, 0:64],
                            tp[:].rearrange("p (g c) -> p g c", c=64),
                        )

                return fn

            pieces.append((b, C_DMA, dma_piece(0, first=True, split=(b == 0))))
            for tqb in range(N_TQB):
                nxt = dma_piece(tqb + 1) if tqb + 1 < N_TQB else None
                pieces.append((b, C_PROJ_GROUP, proj_group(tqb, 0, nxt)))
                pieces.append((b, C_PROJ_GROUP, proj_group(tqb, 1)))
                pieces.append((b, C_PROJ_GROUP, proj_group(tqb, 2)))
                pieces.append((b, C_VTRANS, vtrans_piece(tqb)))
            return (qT, kT, vaug), pieces

        # ---------------- out projection (deferred slices) ----------------
        def queue_outproj(row0, ctx_pack, reserve=False):
            def slice_fn(s):
                def fn():
                    osb = osb_pool.tile([128, D], MDT, tag="osb")
                    for nb in range(2):
                        pso = pj_ps.tile([128, TQB], F32, tag="pj")
                        nc.tensor.matmul(
                            pso[:],
                            ctx_pack[:, s * 128 : (s + 1) * 128],
                            wout_sb[:, nb * TQB : (nb + 1) * TQB],
                            start=True,
                            stop=True,
                        )
                        # both evictions on DVE: ACT is the exp-cadence
                        # engine and its queue is better kept clear
                        nc.vector.tensor_copy(
                            osb[:, nb * TQB : (nb + 1) * TQB], pso[:]
                        )
                    row = row0 + s * 128
                    nc.sync.dma_start(out=out[row : row + 128, :], in_=osb[:])

                return fn

            for s in range(TQB // 128):
                if reserve:
                    reserve_q.append((C_OUTPROJ_SLICE, slice_fn(s)))
                else:
                    outp_q.append(
                        (it_count[0] + OUTPROJ_DELAY, C_OUTPROJ_SLICE, slice_fn(s))
                    )

        # ---------------- main schedule ----------------
        qkv0, pieces0 = make_proj_pieces(0)
        # all q weight columns first (256KB, ~0.7us): the whole first
        # projection group can then run as soon as the first xt half lands
        wr0 = wqkv.rearrange("(c p) f -> p c f", p=128)
        nc.sync.dma_start(
            out=wqkv_sb[:, :, 0:FEATS], in_=wr0[:, :, 0:FEATS]
        )
        pieces0[0][2]()  # xt(b0, tqb0) DMA + vaug ones memsets
        emit_const_dmas()
        for _, _, fn in pieces0[1:]:
            fn()
        qkv_cur = qkv0

        # total attention iterations left (for pacing)
        iters_left = B * sum((tqb + 1) * (TQB // 128) for tqb in range(N_TQB))

        for b in range(B):
            t0 = b * T
            qT, kT, vaug = qkv_cur
            flush_proj(b)
            if DBG and b == 0:
                nc.sync.dma_start(out=dbg_qT[:], in_=qT[:])
                nc.sync.dma_start(out=dbg_kT[:], in_=kT[:])
                nc.sync.dma_start(
                    out=dbg_vaug[:], in_=vaug.rearrange("p a b -> p (a b)")
                )
            if b + 1 < B:
                qkv_cur, nxt_pieces = make_proj_pieces(b + 1)
                proj_q.extend(nxt_pieces)

            for tqb in range(N_TQB):
                tq0 = tqb * TQB
                n_tk = (tqb + 1) * (TQB // 128)
                ops_a = o_ps.tile([DH + 1, TQB], F32, tag="o")
                ops_b = o_ps.tile([DH + 1, TQB], F32, tag="o")

                def emit_pv(tk, lo, pt, first, last):
                    # both heads -> rows 0..64 (ctx 0..63, l at 64). Column
                    # subrange [lo:] is exact: P is zero below the diagonal
                    # band, so those columns never change.
                    nc.tensor.matmul(
                        ops_a[:, lo:TQB],
                        vaug[:, tk, 0 : DH + 1],
                        pt[:, 0, lo:TQB],
                        start=first,
                        stop=last,
                        skip_group_check=True,
                    )
                    nc.tensor.matmul(
                        ops_b[:, lo:TQB],
                        vaug[:, tk, DH + 1 : 2 * DH + 2],
                        pt[:, 1, lo:TQB],
                        start=first,
                        stop=last,
                        skip_group_check=True,
                    )

                pend = []  # two tiles behind: S/exp/tri run ahead of PV
                for tk in range(n_tk):
                    r = tk - tqb * (TQB // 128)
                    lo = 128 * r if r > 0 else 0
                    sps = s_ps.tile([128, HEADS_PER_CORE, TQB], F32, tag="s")
                    for h in range(HEADS_PER_CORE):
                        hp = h * DH
                        nc.tensor.matmul(
                            sps[:, h, lo:TQB],
                            kT[hp : hp + DH, tk * 128 : (tk + 1) * 128],
                            qT[hp : hp + DH, tq0 + lo : tq0 + TQB],
                            start=True,
                            stop=True,
                        )
                    pt = pt_pool.tile([128, HEADS_PER_CORE, TQB], MDT, tag="pt")
                    nc.scalar.activation(
                        pt[:, :, lo:TQB], sps[:, :, lo:TQB], AF.Exp, scale=0.125
                    )
                    if r >= 0:
                        # NOTE: must stay on DVE — running this on gpsimd
                        # costs ~+130us end-to-end (gpsimd serializes badly
                        # inside the exp->PV chain).
                        nc.vector.tensor_tensor(
                            pt[:, :, lo : lo + 128],
                            pt[:, :, lo : lo + 128],
                            tri_sb[:]
                            .unsqueeze(1)
                            .broadcast_to([128, HEADS_PER_CORE, 128]),
                            op=mybir.AluOpType.mult,
                        )
                    if DBG and b == 0 and tqb == 0 and tk == 0:
                        nc.sync.dma_start(
                            out=dbg_pt[:], in_=pt.rearrange("p a b -> p (a b)")
                        )
                    pend.append((tk, lo, pt))
                    if len(pend) > 2:
                        a0 = pend.pop(0)
                        emit_pv(*a0, a0[0] == 0, False)
                    iters_left -= 1
                    it_count[0] += 1
                    if iters_left > 0:
                        pump_step(queued_cost() / iters_left)
                for a0 in pend:
                    emit_pv(*a0, a0[0] == 0, a0[0] == n_tk - 1)

                # -------- epilogue part 1 (inline): evict PV psum + 1/l --
                # (reciprocal/custom-DVE must NOT read PSUM: on HW it reads
                # the wrong bank. Evict to SBUF first — this also frees the
                # psum banks for the next block's PV immediately.)
                ctx_pack = ctx_pool.tile([128, TQB], MDT, tag="ctx")
                osbA = lr_pool.tile([DH + 1, TQB], F32, tag="ev")
                osbB = lr_pool.tile([DH + 1, TQB], F32, tag="ev")
                # both evictions on DVE: with the 1/l chain deferred into
                # e2, the ~1.4us burst still clears before the next block's
                # first tri-mask, and ACT stays exp-only
                nc.vector.tensor_copy(osbA[:], ops_a[:])
                nc.vector.tensor_copy(osbB[:], ops_b[:])

                # -------- epilogue part 2 (deferred): bc, scale, shift --
                # Runs a couple of iterations into the next block so the
                # gpsimd broadcasts never head-of-line block the next
                # block's tri masks (which feed the PV chain).
                def e2(b=b, tqb=tqb, t0=t0, tq0=tq0, osbA=osbA, osbB=osbB,
                       ctx_pack=ctx_pack):
                    # 1/l chain lives here (not at block end) so the DVE
                    # FIFO isn't busy right when the next block's tri-mask
                    # multiplies need it to feed the PV chain
                    lsbA = lr_pool.tile([1, TQB], F32, tag="lsb")
                    lsbB = lr_pool.tile([1, TQB], F32, tag="lsb")
                    nc.vector.tensor_copy(lsbA[:], osbA[DH : DH + 1, :])
                    nc.vector.tensor_copy(lsbB[:], osbB[DH : DH + 1, :])
                    lrA = lr_pool.tile([1, TQB], F32, tag="lr")
                    lrB = lr_pool.tile([1, TQB], F32, tag="lr")
                    nc.vector.reciprocal_approx_fast(lrA[:], lsbA[:])
                    nc.vector.reciprocal_approx_fast(lrB[:], lsbB[:])
                    bcA = bc_pool.tile([DH, TQB], F32, tag="bc")
                    bcB = bc_pool.tile([DH, TQB], F32, tag="bc")
                    nc.gpsimd.partition_broadcast(bcA[:], lrA[:])
                    nc.gpsimd.partition_broadcast(bcB[:], lrB[:])
                    nc.vector.tensor_tensor(
                        ctx_pack[0:DH, :],
                        osbA[0:DH, :],
                        bcA[:],
                        op=mybir.AluOpType.mult,
                    )
                    ctx_b = ctx_pool.tile([DH, TQB], MDT, tag="ctxb")
                    nc.vector.tensor_tensor(
                        ctx_b[:],
                        osbB[0:DH, :],
                        bcB[:],
                        op=mybir.AluOpType.mult,
                    )
                    if SHIFT_DVE:
                        nc.vector.tensor_copy(ctx_pack[DH:128, :], ctx_b[:])
                    else:
                        nc.sync.dma_start(
                            out=ctx_pack[DH:128, :], in_=ctx_b[:]
                        )
                    if DBG and b == 0 and tqb == 0:
                        nc.sync.dma_start(out=dbg_ops[:, 0:TQB], in_=osbA[:])
                        nc.sync.dma_start(
                            out=dbg_ops[:, TQB : 2 * TQB], in_=osbB[:]
                        )
                        nc.sync.dma_start(out=dbg_lr[0:1, :], in_=lrA[:])
                        nc.sync.dma_start(out=dbg_lr[1:2, :], in_=lrB[:])
                        nc.sync.dma_start(out=dbg_ctx[:], in_=ctx_pack[:])
                    queue_outproj(
                        t0 + tq0, ctx_pack, reserve=(b == B - 1 and tqb < 2)
                    )

                outp_q.append((it_count[0] + 2, 100.0, e2))

        # tail: run the final E2 first (it queues the last block's slices),
        # then alternate reserved and final slices so psum-bank reuse waits
        # are covered by independent matmuls
        while outp_q or proj_q or reserve_q:
            if outp_q:
                outp_q.pop(0)[2]()
            if reserve_q:
                reserve_q.pop(0)[1]()
            elif not outp_q and proj_q:
                proj_q.pop(0)[2]()

    nc.finalize()
    return nc


_NC_CACHE = {}


def _mm_dtype():
    name = os.environ.get("KDT", "bf16")
    return {"bf16": mybir.dt.bfloat16, "f32r": mybir.dt.float32r}[name]


def _get_nc():
    key = os.environ.get("KDT", "bf16")
    if key not in _NC_CACHE:
        _NC_CACHE[key] = build_kernel(_mm_dtype())
    return _NC_CACHE[key]


def _make_in_maps(x, W_qkv, W_out):
    npdt = mybir.dt.np(_mm_dtype())
    x2 = np.ascontiguousarray(x.reshape(BT, D).T).astype(npdt)  # (1024, 8192)
    tri = np.triu(np.ones((128, 128))).astype(npdt)
    ident = np.eye(128).astype(npdt)
    in_maps = []
    for c in range(N_CORES):
        wq = W_qkv[:, c * FEATS : (c + 1) * FEATS]
        wk = W_qkv[:, D + c * FEATS : D + (c + 1) * FEATS]
        wv = W_qkv[:, 2 * D + c * FEATS : 2 * D + (c + 1) * FEATS]
        wqkv_c = np.ascontiguousarray(
            np.concatenate([wq, wk, wv], axis=1)
        ).astype(npdt)
        wout_c = np.ascontiguousarray(
            W_out[c * FEATS : (c + 1) * FEATS, :]
        ).astype(npdt)
        in_maps.append(
            {"x_t": x2, "wqkv": wqkv_c, "wout": wout_c, "tri": tri, "ident": ident}
        )
    return in_maps


def run(x, W_qkv, W_out, trace=False, trace_kwargs=None):
    nc = _get_nc()
    in_maps = _make_in_maps(np.asarray(x), np.asarray(W_qkv), np.asarray(W_out))
    res = run_bass_kernel_spmd(
        nc,
        in_maps,
        core_ids=list(range(N_CORES)),
        trace=trace,
        **(trace_kwargs or {}),
    )
    partials = np.stack(
        [res.results[c]["out"].astype(np.float32) for c in range(N_CORES)]
    )
    if os.environ.get("NORMS", "0") == "1":
        for c in range(N_CORES):
            p = partials[c]
            print(
                f"core {c}: norm {np.linalg.norm(p):.3e} max {np.abs(p).max():.3e}"
            )
    full = partials.sum(axis=0, dtype=np.float32).reshape(B, T, D)
    return full, res


def kernel(x, W_qkv, W_out):
    full, _ = run(x, W_qkv, W_out, trace=False)
    return full

